# revision 60
# baseline (speedup 1.0000x reference)
"""AmberDynamics (5-link biped manipulator dynamics) Trainium2 kernel.

Math: per sample, out[0:5] = qdot and out[5:10] = D^{-1} (B u - H) with
D = 2 I + 0.3 (c c^T + s s^T)  (c = cos q, s = sin q).  Woodbury gives a
per-sample 2x2 solve in the (c, s) basis:
  x = e - p*c - q'*s,   e = (B u - 0.05 qd)/2
  p  = (N22*ae - gcs*b2)/det      ae = c.e,  be = s.e
  q' = (N11*b2 - gcs*ae)/det      b2 = be + (20/3)*m,  m = 0.05*v2 + 4.9
  N11 = gcc + 20/3,  N22 = (5 - gcc) + 20/3,  det = N11*N22 - gcs^2
The m*s term of r' = e - m*s and the a/b corrections fold entirely into
b2 (since N22 - gss = 20/3), so the per-sample scalar chain is short and
q+m never needs materializing.

Layout per core: 125000 samples as [125 partitions, 1000 samples], planar
(component-major) fp16 work tiles (VectorE 2x mode; tensor_scalar runs in
4x mode).  Work is split across all three elementwise engines to balance
against the 12 MB/core DMA roofline (33.4 us at the modeled 360 GB/s):
  ScalarE: sin/abs planes, qd^2 / c^2 squares (tile 0: on DVE instead,
    shortening the head-critical Act chain), u-scale, qdot copy
  VectorE: products, sum trees, 2x2 solve chain (fp16 det + fp16
    reciprocal, HW-verified at ~1.4e-3 rel), reconstruction
  GpSimd:  qd scale, c*s product, final sub w/ fused fp16->f32 AoS out
Trig: ScalarE Sin is applied directly to q (|q|<=5.3; table error beyond
+-4.5 affects ~4e-4 of samples at <0.4 abs — negligible in the norm), and
cos q = sin(pi/2 - |q|) keeps its argument inside the table everywhere.
Per-sample scalars broadcast over components via stride-0 views, keeping
packed [P,2,5,T] products in single instructions.
"""

import math

import numpy as np

import concourse.bass as bass
import concourse.bacc as bacc
import concourse.mybir as mybir
from concourse import tile
from concourse.bass_utils import run_bass_kernel_spmd

N_CORES = 8
B_TOTAL = 1_000_000
B_CORE = B_TOTAL // N_CORES  # 125000
P = 125                      # SBUF partitions used (125*1000 = 125000)
SPP = B_CORE // P            # samples per partition = 1000
SIZES = [130, 238, 242, 240, 150]
F32 = mybir.dt.float32
F16 = mybir.dt.float16
PI_2 = math.pi / 2.0
K3 = 20.0 / 3.0
Sin = mybir.ActivationFunctionType.Sin
Abs = mybir.ActivationFunctionType.Abs
Square = mybir.ActivationFunctionType.Square
Copy = mybir.ActivationFunctionType.Copy
MUL = mybir.AluOpType.mult
ADD = mybir.AluOpType.add


def build_bass() -> bass.Bass:
    nc = bacc.Bacc()
    # register pi/2 so activation(..., Sin, bias=PI_2) can resolve a const AP
    _pi2 = nc.alloc_sbuf_tensor("const-f32-pi2", [128, 1], F32)
    nc.gpsimd.memset(_pi2.ap(), PI_2)
    nc.const_aps.aps[(F32, PI_2)] = _pi2.ap()
    state = nc.declare_dram_parameter("state", [B_CORE, 10], F32, isOutput=False)
    u_in = nc.declare_dram_parameter("u", [B_CORE, 4], F32, isOutput=False)
    out = nc.declare_dram_parameter("out", [B_CORE, 10], F32, isOutput=True)

    st3 = state[:].rearrange("(p t) c -> p t c", p=P)   # [125, 1000, 10]
    u3 = u_in[:].rearrange("(p t) c -> p t c", p=P)     # [125, 1000, 4]
    out3 = out[:].rearrange("(p t) c -> p t c", p=P)    # [125, 1000, 10]

    from contextlib import ExitStack

    with tile.TileContext(nc) as tc, ExitStack() as ctx:
        pool = ctx.enter_context(tc.tile_pool(name="io", bufs=3))
        wk = ctx.enter_context(tc.tile_pool(name="work", bufs=3))
        sc = ctx.enter_context(tc.tile_pool(name="scalars", bufs=3))

        # prime the Sin/Abs/Square table before the loop
        warm = sc.tile([P, 1], F32, tag="warm")
        nc.scalar.activation(warm[:], _pi2.ap()[0:P], Sin)
        nc.scalar.activation(warm[:], _pi2.ap()[0:P], Abs)

        starts = [sum(SIZES[:i]) for i in range(len(SIZES))]
        nlast = len(SIZES) - 1
        for it, Tt in enumerate(SIZES):
            ts = slice(starts[it], starts[it] + Tt)

            ST = pool.tile([P, Tt, 10], F32, tag="ST")
            UT = pool.tile([P, Tt, 4], F32, tag="UT")
            if it == 0:
                h0 = Tt // 3
                nc.sync.dma_start(out=ST[:, 0:h0, :], in_=st3[:, ts][:, 0:h0, :])
                nc.sync.dma_start(out=UT[:], in_=u3[:, ts, :])
                nc.sync.dma_start(out=ST[:, h0:Tt, :], in_=st3[:, ts][:, h0:Tt, :])
            else:
                nc.sync.dma_start(out=ST[:], in_=st3[:, ts, :])
                nc.sync.dma_start(out=UT[:], in_=u3[:, ts, :])

            Qv = ST[:, :, 0:5].rearrange("p t c -> p c t")    # [125,5,T] f32
            QDv = ST[:, :, 5:10].rearrange("p t c -> p c t")  # [125,5,T] f32
            OUT = pool.tile([P, Tt, 10], F32, tag="OUT")

            # ---- trig (ScalarE) + early feeders ----
            CS2 = wk.tile([P, 2, 5, Tt], F16, tag="CS2")
            C = CS2[:, 0, :, :]
            S = CS2[:, 1, :, :]
            PCQS = wk.tile([P, 2, 5, Tt], F16, tag="PCQS")
            AQ = PCQS[:, 0, :, :]  # early-phase scratch, reused for recon later
            # PR product slots: (c2, cs, qsq, ce, se)
            PR = wk.tile([P, 5, 5, Tt], F16, tag="PR")
            E = wk.tile([P, 5, Tt], F16, tag="E")
            XU = wk.tile([P, 5, Tt], F16, tag="XU")
            US = XU[:, 0:4, :]  # early-phase alias; full XU reused as X1 later

            tr_slices = (
                [slice(0, Tt // 3), slice(Tt // 3, Tt)]
                if (it == 0 and Tt >= 100)
                else [slice(0, Tt)]
            )
            if it != 0:
                nc.scalar.activation(US[:], UT[:].rearrange("p t c -> p c t"), Copy, scale=0.5)
            for j, sl in enumerate(tr_slices):
                # S = sin(q) directly; C = sin(pi/2 - |q|)
                nc.scalar.activation(CS2[:, 1, :, sl], Qv[:, :, sl], Sin)
                nc.scalar.activation(AQ[:, :, sl], Qv[:, :, sl], Abs)
                nc.scalar.activation(CS2[:, 0, :, sl], AQ[:, :, sl], Sin, scale=-1.0, bias=PI_2)
                # E = -0.025*qd (GpSimd)
                nc.gpsimd.tensor_scalar(E[:, :, sl], QDv[:, :, sl], -0.025, None, MUL)
            if it == 0:
                # tile 0: scale u only after the full trig chain — cos is the
                # head critical path and must not queue behind the UT DMA
                nc.scalar.activation(US[:], UT[:].rearrange("p t c -> p c t"), Copy, scale=0.5)
            if it == 0:
                # tile 0: qsq = (0.025*qd)^2 from E0 and c^2 = C*C on DVE,
                # filling its startup gap while ScalarE's serial trig chain
                # (the head critical path) stays 2 ops shorter.  The 6.25e-4
                # scale on v2 folds into this tile's km constant below.
                nc.vector.tensor_mul(out=PR[:, 2, :, :], in0=E[:], in1=E[:])
            else:
                nc.scalar.activation(PR[:, 2, :, :], QDv, Square)  # qsq = qd^2
            nc.vector.tensor_add(out=E[:, 1:5, :], in0=E[:, 1:5, :], in1=US[:])

            # ---- products ----
            if it == 0:
                nc.vector.tensor_mul(out=PR[:, 0, :, :], in0=C, in1=C)   # c^2
            else:
                nc.scalar.activation(PR[:, 0, :, :], C, Square)          # c^2
            nc.gpsimd.tensor_mul(out=PR[:, 1, :, :], in0=C, in1=S)   # c*s
            Eb = E[:].rearrange("p (o c) t -> p o c t", o=1).broadcast_to([P, 2, 5, Tt])
            nc.vector.tensor_mul(out=PR[:, 3:5, :, :], in0=CS2[:], in1=Eb)  # ce, se

            # qdot passthrough: out[:, 0:5] = qdot (ScalarE f32 copy)
            nc.scalar.activation(
                OUT[:, :, 0:5].rearrange("p t c -> p c t"), QDv, Copy
            )

            # ---- packed trees: G5 = (gcc, gcs, v2, ae, be) ----
            TL1 = wk.tile([P, 5, 2, Tt], F16, tag="TL1")
            G5 = sc.tile([P, 5, Tt], F16, tag="G5")
            prv = PR[:, :, 0:4, :].rearrange("p q (b c) t -> p q b c t", b=2)
            for a, b in ((0, 3), (3, 5)):  # (c2,cs,qsq) tree, then (ce,se)
                nc.vector.tensor_add(
                    out=TL1[:, a:b, :, :], in0=prv[:, a:b, :, 0, :], in1=prv[:, a:b, :, 1, :]
                )
                nc.vector.tensor_add(
                    out=G5[:, a:b, :], in0=TL1[:, a:b, 0, :], in1=TL1[:, a:b, 1, :]
                )
                nc.vector.tensor_add(
                    out=G5[:, a:b, :], in0=G5[:, a:b, :], in1=PR[:, a:b, 4, :]
                )
            gcc = G5[:, 0, :]
            gcs = G5[:, 1, :]
            v2 = G5[:, 2, :]
            ae = G5[:, 3, :]
            be = G5[:, 4, :]  # becomes b2 in place

            # ---- scalar chain ([P,T] planes) ----
            km = sc.tile([P, Tt], F16, tag="km")
            # k*m = (20/3)*(0.05*v2 + 4.9) = v2/3 + 98/3
            kmv = (1600.0 / 3.0) if it == 0 else (1.0 / 3.0)
            nc.vector.tensor_scalar(km[:], v2, kmv, 98.0 / 3.0, MUL, ADD)
            nc.vector.tensor_add(out=be, in0=be, in1=km[:])  # b2
            NN = sc.tile([P, 2, Tt], F16, tag="NN")  # (N22, N11)
            nc.vector.tensor_scalar(NN[:, 0, :], gcc, -1.0, 5.0 + K3, MUL, ADD)
            nc.vector.tensor_scalar(NN[:, 1, :], gcc, 1.0, K3, MUL, ADD)
            DT1 = sc.tile([P, Tt], F16, tag="DT1")
            DT2 = sc.tile([P, Tt], F16, tag="DT2")
            det = sc.tile([P, Tt], F16, tag="det")
            inv16 = sc.tile([P, Tt], F16, tag="inv16")
            nc.vector.tensor_mul(out=DT1[:], in0=NN[:, 0, :], in1=NN[:, 1, :])
            nc.vector.tensor_mul(out=DT2[:], in0=gcs, in1=gcs)
            nc.vector.tensor_sub(out=det[:], in0=DT1[:], in1=DT2[:])
            # det in [~44, 136]; f16 reciprocal is ~1.4e-3 rel — well inside
            # the 2e-2 norm budget (HW-verified)
            with nc.allow_low_precision(reason="f16 1/det: 1.4e-3 rel, budget 2e-2"):
                nc.vector.reciprocal(out=inv16[:], in_=det[:])

            # T1 = (N22*ae, N11*b2); T2 = (gcs*b2, gcs*ae); num = T1 - T2
            T1 = sc.tile([P, 2, Tt], F16, tag="T1")
            T2 = sc.tile([P, 2, Tt], F16, tag="T2")
            PQ = sc.tile([P, 2, Tt], F16, tag="PQ")
            nc.vector.tensor_mul(out=T1[:], in0=NN[:], in1=G5[:, 3:5, :])
            gb2 = G5[:, 1:2, :].broadcast_to([P, 2, Tt])
            nc.vector.tensor_mul(out=T2[:], in0=gb2, in1=G5[:, 4:2:-1, :])
            nc.vector.tensor_sub(out=T1[:], in0=T1[:], in1=T2[:])
            ib = inv16[:].rearrange("p (o t) -> p o t", o=1).broadcast_to([P, 2, Tt])
            nc.vector.tensor_mul(out=PQ[:], in0=T1[:], in1=ib)

            # ---- recon: x = e - p*c - q'*s ----
            pqb = PQ[:].rearrange("p w (o t) -> p w o t", o=1).broadcast_to([P, 2, 5, Tt])
            X1 = XU  # full [P,5,T]; US alias already consumed
            last = it == nlast
            if not last:
                nc.vector.tensor_mul(out=PCQS[:], in0=pqb, in1=CS2[:])
                # split the e - p*c sub 3/2 across VectorE/GpSimd
                nc.vector.tensor_sub(
                    out=X1[:, 0:3, :], in0=E[:, 0:3, :], in1=PCQS[:, 0, 0:3, :]
                )
                nc.gpsimd.tensor_sub(
                    out=X1[:, 3:5, :], in0=E[:, 3:5, :], in1=PCQS[:, 0, 3:5, :]
                )
            # final sub w/ fused fp16->f32 strided out, chunked so the out DMA
            # starts early; the last tile pipelines recon per chunk on VectorE
            # to shrink the kernel tail
            Xv = OUT[:, :, 5:10].rearrange("p t c -> p c t")
            nch = 3 if last else 2
            step = Tt // nch
            bounds = [j * step for j in range(nch)] + [Tt]
            for lo, hi in zip(bounds[:-1], bounds[1:]):
                if last:
                    nc.vector.tensor_mul(
                        out=PCQS[:, :, :, lo:hi], in0=pqb[:, :, :, lo:hi],
                        in1=CS2[:, :, :, lo:hi],
                    )
                    nc.vector.tensor_sub(
                        out=X1[:, :, lo:hi], in0=E[:, :, lo:hi],
                        in1=PCQS[:, 0, :, lo:hi],
                    )
                eng = nc.vector if last else nc.gpsimd
                eng.tensor_sub(
                    out=Xv[:, :, lo:hi],
                    in0=X1[:, :, lo:hi],
                    in1=PCQS[:, 1, :, lo:hi],
                )
                nc.sync.dma_start(
                    out=out3[:, starts[it] + lo : starts[it] + hi, :],
                    in_=OUT[:, lo:hi, :],
                )

    nc.finalize()
    return nc


_NC_CACHE = None


def _get_nc():
    global _NC_CACHE
    if _NC_CACHE is None:
        _NC_CACHE = build_bass()
    return _NC_CACHE


def kernel(t: np.ndarray, state: np.ndarray, u: np.ndarray, _trace: bool = False):
    state = np.ascontiguousarray(np.asarray(state, dtype=np.float32))
    u2 = np.ascontiguousarray(np.asarray(u, dtype=np.float32).reshape(B_TOTAL, 4))
    nc = _get_nc()
    in_maps = [
        {
            "state": state[k * B_CORE : (k + 1) * B_CORE],
            "u": u2[k * B_CORE : (k + 1) * B_CORE],
        }
        for k in range(N_CORES)
    ]
    # the axon-proxied NeuronCores occasionally throw a transient
    # NRT_EXEC_UNIT_UNRECOVERABLE; retry a couple of times before giving up
    last_err = None
    for attempt in range(3):
        try:
            r = run_bass_kernel_spmd(
                nc, in_maps, core_ids=list(range(N_CORES)), trace=_trace
            )
            break
        except Exception as e:
            last_err = e
            if "UNRECOVERABLE" not in str(e) and "UNAVAILABLE" not in str(e):
                raise
            import time as _time

            _time.sleep(15)
            try:
                import jax

                jax.clear_backends()
            except Exception:
                pass
    else:
        raise last_err
    full = np.concatenate([r.results[k]["out"] for k in range(N_CORES)], axis=0)
    out = full.reshape(B_TOTAL, 10, 1)
    if _trace:
        return out, r
    return out

